# revision 21
# baseline (speedup 1.0000x reference)
"""MoE router (nn_BaseRouter) Trainium2 Bass kernel.

Problem: router MLP (Linear->ReLU->Linear) + softmax + top-2 over 8 experts,
producing dispatch/combine tensors [B,S,E,CAP] where only capacity slot 0 is
ever nonzero, plus router_probs and a scalar aux loss.

Sharding: data-parallel over the B*S=4096 tokens -> 512 tokens per core on
8 cores; the tiny router weights are replicated. The host hands each core
pre-transposed operands (x^T shard, w1^T, w2^T) so the device does no
layout work, and concatenates the per-core output shards.

The memory roofline is the ~50MB/core of (mostly zero) dispatch/combine
output. The zero regions (capacity slots 1..1535, 87% of all bytes) have no
data dependency at all: they stream out from a single read-only zero SBUF
tile starting at t=0 on the SP HWDGE ring. The compute chain only gates the
tiny slot-0 scatter DMAs (1024 x 4B strided elements per block), which go on
the ACT HWDGE ring together with the input loads, so nothing queues behind
the 50MB zero stream.
"""

import sys

import numpy as np

if "/opt/trn_rl_repo" not in sys.path:
    sys.path.insert(0, "/opt/trn_rl_repo")

import concourse.bacc as bacc
import concourse.bass as bass
import concourse.mybir as mybir
import concourse.tile as tile
from concourse import bass_utils

B, S, HID = 2, 2048, 1024
E, CAP = 8, 1536
NCORES = 8
TOK = B * S            # 4096 tokens
TPC = TOK // NCORES    # 512 tokens per core
P = 128                # partitions
NT = TPC // P          # 4 token tiles per core
KT = HID // P          # 8 contraction tiles
ROW = E * CAP          # 12288 floats per token in dispatch/combine
F32 = mybir.dt.float32

# Set by test harness to capture profiling info from the last run.
TRACE = False
LAST_RESULT = None


def build_bass() -> bass.Bass:
    # Bacc (not raw Bass): its compile() legalizes multi-wait instructions
    # (move_matmul_waits_to_ldweights / generate_event_semaphores) which the
    # walrus codegen requires.
    nc = bacc.Bacc("TRN2", debug=False, num_devices=NCORES,
                   enable_partition_id=False)

    xt_d = nc.dram_tensor("xt", [HID, TPC], F32, kind="ExternalInput").ap()
    w1t_d = nc.dram_tensor("w1t", [HID, HID], F32, kind="ExternalInput").ap()
    b1_d = nc.dram_tensor("b1", [HID], F32, kind="ExternalInput").ap()
    w2t_d = nc.dram_tensor("w2t", [HID, E], F32, kind="ExternalInput").ap()
    b2_d = nc.dram_tensor("b2", [E], F32, kind="ExternalInput").ap()
    disp = nc.dram_tensor("disp", [TPC, ROW], F32, kind="ExternalOutput").ap()
    comb = nc.dram_tensor("comb", [TPC, ROW], F32, kind="ExternalOutput").ap()
    probs = nc.dram_tensor("probs", [TPC, E], F32, kind="ExternalOutput").ap()

    disp3 = disp.rearrange("t (e c) -> t e c", e=E)
    comb3 = comb.rearrange("t (e c) -> t e c", e=E)

    with tile.TileContext(nc) as tc:
        with (
            tc.tile_pool(name="zrow", bufs=1) as zrow_pool,
            tc.tile_pool(name="const", bufs=1) as const_pool,
            tc.tile_pool(name="wt", bufs=1) as wt_pool,
            tc.tile_pool(name="acts", bufs=1) as acts_pool,
            tc.tile_pool(name="small", bufs=4) as small_pool,
            tc.tile_pool(name="psum_m", bufs=4, space="PSUM") as psum_m_pool,
            tc.tile_pool(name="psum_l", bufs=2, space="PSUM") as psum_l_pool,
        ):
            # ---- zero stream: no compute dependency, issued first on the
            # SP ring so it saturates HBM from t~0. Skips capacity slot 0
            # (6140B descriptor rows), so it is fully disjoint from the
            # slot-0 value scatters — no cross-DMA ordering needed (Tile
            # does not track DRAM write-write overlap, so disjointness is
            # load-bearing). The source replicates one small zero tile
            # across the 8 expert rows with a step-0 AP dim, so only 1535
            # floats per partition need zeroing before the stream starts.
            zrow = zrow_pool.tile([P, CAP], F32)
            nc.vector.memset(zrow, 0.0)
            zsrc = bass.AP(tensor=zrow.tensor, offset=zrow.offset + 1,
                           ap=[zrow.ap[0], [0, E], [1, CAP - 1]])
            for m in range(NT):
                nc.sync.dma_start(out=disp3[m * P:(m + 1) * P, :, 1:],
                                  in_=zsrc)
                nc.sync.dma_start(out=comb3[m * P:(m + 1) * P, :, 1:],
                                  in_=zsrc)

            # ---- input loads on the ACT ring (keep the SP ring exclusively
            # for the zero stream).
            # b1 laid out [o%128, o//128]: column j = per-partition bias of
            # output tile j of matmul 1.
            b1_sb = const_pool.tile([P, KT], F32)
            nc.scalar.dma_start(out=b1_sb, in_=b1_d.rearrange("(a b) -> b a",
                                                              b=P))
            # b2 replicated across partitions.
            b2_sb = const_pool.tile([P, E], F32)
            b2_bcast = bass.AP(tensor=b2_d.tensor, offset=b2_d.offset,
                               ap=[[0, P], b2_d.ap[0]])
            nc.scalar.dma_start(out=b2_sb, in_=b2_bcast)

            # Fused single-DMA loads (one per tensor): keeps the total HWDGE
            # DMA count low enough (~2 per completion-sem lane) that Tile's
            # per-lane in-flight throttle never makes a zero-stream DMA wait
            # on an input/scatter completion (that cross-stream coupling
            # fully stalled the zero stream twice per run).
            w2t_sb = const_pool.tile([P, KT, E], F32)
            nc.scalar.dma_start(out=w2t_sb,
                                in_=w2t_d.rearrange("(i p) e -> p i e", p=P))

            xt_sb = acts_pool.tile([P, KT, TPC], F32, tag="xt")
            nc.scalar.dma_start(out=xt_sb,
                                in_=xt_d.rearrange("(i p) t -> p i t", p=P))

            w1t_sb = wt_pool.tile([P, KT, HID], F32)
            nc.scalar.dma_start(out=w1t_sb,
                                in_=w1t_d.rearrange("(i p) o -> p i o", p=P))

            # ---- matmul 1: h^T[o,t] = relu(w1t.T @ x^T + b1), kept
            # transposed so it feeds matmul 2 as lhsT directly.
            ht = acts_pool.tile([P, KT, TPC], F32, tag="ht")
            for j in range(KT):
                pm = psum_m_pool.tile([P, TPC], F32, tag="pm")
                for i in range(KT):
                    nc.tensor.matmul(pm, lhsT=w1t_sb[:, i, j * P:(j + 1) * P],
                                     rhs=xt_sb[:, i, :],
                                     start=(i == 0), stop=(i == KT - 1))
                nc.scalar.activation(out=ht[:, j, :], in_=pm,
                                     func=mybir.ActivationFunctionType.Relu,
                                     bias=b1_sb[:, j:j + 1], scale=1.0)

            # ---- per 128-token block: logits, softmax, top-2, slot-0 values
            for m in range(NT):
                pl = psum_l_pool.tile([P, E], F32, tag="pl")
                for i in range(KT):
                    nc.tensor.matmul(pl, lhsT=ht[:, i, m * P:(m + 1) * P],
                                     rhs=w2t_sb[:, i, :],
                                     start=(i == 0), stop=(i == KT - 1))
                logits = small_pool.tile([P, E], F32, tag="logits")
                nc.vector.tensor_add(out=logits, in0=pl, in1=b2_sb)

                # softmax over the 8 experts
                nmax = small_pool.tile([P, 1], F32, tag="nmax")
                nc.vector.reduce_max(out=nmax, in_=logits,
                                     axis=mybir.AxisListType.X, negate=True)
                exps = small_pool.tile([P, E], F32, tag="exps")
                ssum = small_pool.tile([P, 1], F32, tag="ssum")
                nc.scalar.activation(out=exps, in_=logits,
                                     func=mybir.ActivationFunctionType.Exp,
                                     bias=nmax, scale=1.0, accum_out=ssum)
                rsum = small_pool.tile([P, 1], F32, tag="rsum")
                nc.vector.reciprocal(rsum, ssum)
                pr = small_pool.tile([P, E], F32, tag="pr")
                nc.vector.tensor_scalar_mul(pr, exps, rsum)
                nc.scalar.dma_start(out=probs[m * P:(m + 1) * P, :], in_=pr)

                # top-2: max8 gives the row's 8 maxima in descending order;
                # match_replace consumes one match per slot (top_k tie
                # semantics). Slots 2..7 are set to -1 so they never match.
                maxes = small_pool.tile([P, 8], F32, tag="maxes")
                nc.vector.max(out=maxes, in_=pr)
                den = small_pool.tile([P, 1], F32, tag="den")
                nc.vector.tensor_add(out=den, in0=maxes[:, 0:1],
                                     in1=maxes[:, 1:2])
                rden = small_pool.tile([P, 1], F32, tag="rden")
                nc.vector.reciprocal(rden, den)
                nc.vector.memset(maxes[:, 2:8], -1.0)
                scr = small_pool.tile([P, E], F32, tag="scr")
                nc.vector.match_replace(out=scr, in_to_replace=maxes,
                                        in_values=pr, imm_value=0.0)
                diff = small_pool.tile([P, E], F32, tag="diff")
                nc.vector.tensor_sub(out=diff, in0=pr, in1=scr)
                disp_v = small_pool.tile([P, E], F32, tag="disp_v")
                nc.vector.tensor_scalar(out=disp_v, in0=diff, scalar1=0.0,
                                        scalar2=None,
                                        op0=mybir.AluOpType.is_gt)
                comb_v = small_pool.tile([P, E], F32, tag="comb_v")
                nc.vector.tensor_scalar_mul(comb_v, diff, rden)

                # slot-0 scatter: 1024 strided 4B elements per tensor,
                # disjoint from the zero stream (which skips slot 0).
                nc.scalar.dma_start(out=disp3[m * P:(m + 1) * P, :, 0:1],
                                    in_=disp_v.unsqueeze(2))
                nc.scalar.dma_start(out=comb3[m * P:(m + 1) * P, :, 0:1],
                                    in_=comb_v.unsqueeze(2))

    nc.compile()
    return nc


def kernel(**inputs) -> tuple:
    global LAST_RESULT
    hs = np.asarray(inputs["hidden_states"], dtype=np.float32)
    w1 = np.asarray(inputs["w1"], dtype=np.float32)
    b1 = np.ascontiguousarray(np.asarray(inputs["b1"], dtype=np.float32))
    w2 = np.asarray(inputs["w2"], dtype=np.float32)
    b2 = np.ascontiguousarray(np.asarray(inputs["b2"], dtype=np.float32))

    xflat = hs.reshape(TOK, HID)
    w1t = np.ascontiguousarray(w1.T)
    w2t = np.ascontiguousarray(w2.T)

    nc = build_bass()
    in_maps = [
        {
            "xt": np.ascontiguousarray(xflat[c * TPC:(c + 1) * TPC].T),
            "w1t": w1t, "b1": b1, "w2t": w2t, "b2": b2,
        }
        for c in range(NCORES)
    ]
    res = bass_utils.run_bass_kernel_spmd(nc, in_maps,
                                          core_ids=list(range(NCORES)),
                                          trace=TRACE)
    LAST_RESULT = res

    disp = np.concatenate([np.asarray(r["disp"]) for r in res.results],
                          axis=0).reshape(B, S, E, CAP)
    comb = np.concatenate([np.asarray(r["comb"]) for r in res.results],
                          axis=0).reshape(B, S, E, CAP)
    probs = np.concatenate([np.asarray(r["probs"]) for r in res.results],
                           axis=0).reshape(B, S, E)

    pbar = probs.reshape(-1, E).astype(np.float32).mean(axis=0)
    aux = np.float32(np.sum(pbar * np.log(pbar * np.float32(E) +
                                          np.float32(1e-9)),
                            dtype=np.float32))
    return disp, comb, probs, aux


# revision 25
# speedup vs baseline: 1.4928x; 1.4928x over previous
"""MoE router (nn_BaseRouter) Trainium2 Bass kernel.

Problem: router MLP (Linear->ReLU->Linear) + softmax + top-2 over 8 experts,
producing dispatch/combine tensors [B,S,E,CAP] where only capacity slot 0 is
ever nonzero, plus router_probs and a scalar aux loss.

Sharding: data-parallel over the B*S=4096 tokens -> 512 tokens per core on
8 cores; the tiny router weights are replicated. The host hands each core
pre-transposed operands (x^T shard, w1^T, w2^T) so the device does no
layout work, and concatenates the per-core output shards.

The memory roofline is the ~50MB/core of (mostly zero) dispatch/combine
output. The zero regions (capacity slots 1..1535, 87% of all bytes) have no
data dependency at all: they stream out from a single read-only zero SBUF
tile starting at t=0 on the SP HWDGE ring. The compute chain only gates the
tiny slot-0 scatter DMAs (1024 x 4B strided elements per block), which go on
the ACT HWDGE ring together with the input loads, so nothing queues behind
the 50MB zero stream.
"""

import sys

import numpy as np

if "/opt/trn_rl_repo" not in sys.path:
    sys.path.insert(0, "/opt/trn_rl_repo")

import concourse.bacc as bacc
import concourse.bass as bass
import concourse.mybir as mybir
import concourse.tile as tile
from concourse import bass_utils

B, S, HID = 2, 2048, 1024
E, CAP = 8, 1536
NCORES = 8
TOK = B * S            # 4096 tokens
TPC = TOK // NCORES    # 512 tokens per core
P = 128                # partitions
NT = TPC // P          # 4 token tiles per core
KT = HID // P          # 8 contraction tiles
ROW = E * CAP          # 12288 floats per token in dispatch/combine
F32 = mybir.dt.float32

# Set by test harness to capture profiling info from the last run.
TRACE = False
LAST_RESULT = None


def build_bass() -> bass.Bass:
    # Bacc (not raw Bass): its compile() legalizes multi-wait instructions
    # (move_matmul_waits_to_ldweights / generate_event_semaphores) which the
    # walrus codegen requires.
    nc = bacc.Bacc("TRN2", debug=False, num_devices=NCORES,
                   enable_partition_id=False)

    xt_d = nc.dram_tensor("xt", [HID, TPC], F32, kind="ExternalInput").ap()
    w1t_d = nc.dram_tensor("w1t", [HID, HID], F32, kind="ExternalInput").ap()
    b1_d = nc.dram_tensor("b1", [HID], F32, kind="ExternalInput").ap()
    w2t_d = nc.dram_tensor("w2t", [HID, E], F32, kind="ExternalInput").ap()
    b2_d = nc.dram_tensor("b2", [E], F32, kind="ExternalInput").ap()
    disp = nc.dram_tensor("disp", [TPC, ROW], F32, kind="ExternalOutput").ap()
    comb = nc.dram_tensor("comb", [TPC, ROW], F32, kind="ExternalOutput").ap()
    probs = nc.dram_tensor("probs", [TPC, E], F32, kind="ExternalOutput").ap()

    disp3 = disp.rearrange("t (e c) -> t e c", e=E)
    comb3 = comb.rearrange("t (e c) -> t e c", e=E)

    with tile.TileContext(nc) as tc:
        with (
            tc.tile_pool(name="zrow", bufs=1) as zrow_pool,
            tc.tile_pool(name="const", bufs=1) as const_pool,
            tc.tile_pool(name="wt", bufs=1) as wt_pool,
            tc.tile_pool(name="acts", bufs=1) as acts_pool,
            tc.tile_pool(name="small", bufs=4) as small_pool,
            tc.tile_pool(name="psum_m", bufs=4, space="PSUM") as psum_m_pool,
            tc.tile_pool(name="psum_l", bufs=2, space="PSUM") as psum_l_pool,
        ):
            # ---- zero stream: no compute dependency, issued first on the
            # SP ring so it saturates HBM from t~0. Skips capacity slot 0
            # (6140B descriptor rows), so it is fully disjoint from the
            # slot-0 value scatters — no cross-DMA ordering needed. Memset
            # is split across two engines to halve the start latency.
            # Small zero tile; the DMA source replicates it across the 8
            # expert rows with a step-0 AP dim, so only 1535 floats per
            # partition need zeroing before the stream can start.
            zrow = zrow_pool.tile([P, CAP], F32)
            nc.vector.memset(zrow, 0.0)
            zsrc = bass.AP(tensor=zrow.tensor, offset=zrow.offset + 1,
                           ap=[zrow.ap[0], [0, E], [1, CAP - 1]])
            for m in range(NT):
                nc.sync.dma_start(out=disp3[m * P:(m + 1) * P, :, 1:],
                                  in_=zsrc)
                nc.sync.dma_start(out=comb3[m * P:(m + 1) * P, :, 1:],
                                  in_=zsrc)

            # ---- input loads on the ACT ring (keep the SP ring exclusively
            # for the zero stream).
            # b1 laid out [o%128, o//128]: column j = per-partition bias of
            # output tile j of matmul 1.
            b1_sb = const_pool.tile([P, KT], F32)
            nc.gpsimd.dma_start(out=b1_sb, in_=b1_d.rearrange("(a b) -> b a",
                                                              b=P))
            # b2 replicated across partitions.
            b2_sb = const_pool.tile([P, E], F32)
            b2_bcast = bass.AP(tensor=b2_d.tensor, offset=b2_d.offset,
                               ap=[[0, P], b2_d.ap[0]])
            nc.gpsimd.dma_start(out=b2_sb, in_=b2_bcast)

            w2t_sb = const_pool.tile([P, KT, E], F32)
            for i in range(KT):
                nc.gpsimd.dma_start(out=w2t_sb[:, i, :],
                                    in_=w2t_d[i * P:(i + 1) * P, :])

            xt_sb = acts_pool.tile([P, KT, TPC], F32, tag="xt")
            for i in range(KT):
                nc.gpsimd.dma_start(out=xt_sb[:, i, :],
                                    in_=xt_d[i * P:(i + 1) * P, :])

            w1t_sb = wt_pool.tile([P, KT, HID], F32)
            for i in range(KT):
                nc.gpsimd.dma_start(out=w1t_sb[:, i, :],
                                    in_=w1t_d[i * P:(i + 1) * P, :])

            # ---- matmul 1: h^T[o,t] = relu(w1t.T @ x^T + b1), kept
            # transposed so it feeds matmul 2 as lhsT directly.
            ht = acts_pool.tile([P, KT, TPC], F32, tag="ht")
            for j in range(KT):
                pm = psum_m_pool.tile([P, TPC], F32, tag="pm")
                for i in range(KT):
                    nc.tensor.matmul(pm, lhsT=w1t_sb[:, i, j * P:(j + 1) * P],
                                     rhs=xt_sb[:, i, :],
                                     start=(i == 0), stop=(i == KT - 1))
                nc.scalar.activation(out=ht[:, j, :], in_=pm,
                                     func=mybir.ActivationFunctionType.Relu,
                                     bias=b1_sb[:, j:j + 1], scale=1.0)

            # ---- per 128-token block: logits, softmax, top-2, slot-0 values
            for m in range(NT):
                pl = psum_l_pool.tile([P, E], F32, tag="pl")
                for i in range(KT):
                    nc.tensor.matmul(pl, lhsT=ht[:, i, m * P:(m + 1) * P],
                                     rhs=w2t_sb[:, i, :],
                                     start=(i == 0), stop=(i == KT - 1))
                logits = small_pool.tile([P, E], F32, tag="logits")
                nc.vector.tensor_add(out=logits, in0=pl, in1=b2_sb)

                # softmax over the 8 experts
                nmax = small_pool.tile([P, 1], F32, tag="nmax")
                nc.vector.reduce_max(out=nmax, in_=logits,
                                     axis=mybir.AxisListType.X, negate=True)
                exps = small_pool.tile([P, E], F32, tag="exps")
                ssum = small_pool.tile([P, 1], F32, tag="ssum")
                nc.scalar.activation(out=exps, in_=logits,
                                     func=mybir.ActivationFunctionType.Exp,
                                     bias=nmax, scale=1.0, accum_out=ssum)
                rsum = small_pool.tile([P, 1], F32, tag="rsum")
                nc.vector.reciprocal(rsum, ssum)
                pr = small_pool.tile([P, E], F32, tag="pr")
                nc.vector.tensor_scalar_mul(pr, exps, rsum)
                nc.gpsimd.dma_start(out=probs[m * P:(m + 1) * P, :], in_=pr)

                # top-2: max8 gives the row's 8 maxima in descending order;
                # match_replace consumes one match per slot (top_k tie
                # semantics). Slots 2..7 are set to -1 so they never match.
                maxes = small_pool.tile([P, 8], F32, tag="maxes")
                nc.vector.max(out=maxes, in_=pr)
                den = small_pool.tile([P, 1], F32, tag="den")
                nc.vector.tensor_add(out=den, in0=maxes[:, 0:1],
                                     in1=maxes[:, 1:2])
                rden = small_pool.tile([P, 1], F32, tag="rden")
                nc.vector.reciprocal(rden, den)
                nc.vector.memset(maxes[:, 2:8], -1.0)
                scr = small_pool.tile([P, E], F32, tag="scr")
                nc.vector.match_replace(out=scr, in_to_replace=maxes,
                                        in_values=pr, imm_value=0.0)
                diff = small_pool.tile([P, E], F32, tag="diff")
                nc.vector.tensor_sub(out=diff, in0=pr, in1=scr)
                disp_v = small_pool.tile([P, E], F32, tag="disp_v")
                nc.vector.tensor_scalar(out=disp_v, in0=diff, scalar1=0.0,
                                        scalar2=None,
                                        op0=mybir.AluOpType.is_gt)
                comb_v = small_pool.tile([P, E], F32, tag="comb_v")
                nc.vector.tensor_scalar_mul(comb_v, diff, rden)

                # slot-0 scatter: 1024 strided 4B elements per tensor,
                # disjoint from the zero stream (which skips slot 0).
                nc.gpsimd.dma_start(out=disp3[m * P:(m + 1) * P, :, 0:1],
                                    in_=disp_v.unsqueeze(2))
                nc.gpsimd.dma_start(out=comb3[m * P:(m + 1) * P, :, 0:1],
                                    in_=comb_v.unsqueeze(2))

    nc.compile()
    return nc


def kernel(**inputs) -> tuple:
    global LAST_RESULT
    hs = np.asarray(inputs["hidden_states"], dtype=np.float32)
    w1 = np.asarray(inputs["w1"], dtype=np.float32)
    b1 = np.ascontiguousarray(np.asarray(inputs["b1"], dtype=np.float32))
    w2 = np.asarray(inputs["w2"], dtype=np.float32)
    b2 = np.ascontiguousarray(np.asarray(inputs["b2"], dtype=np.float32))

    xflat = hs.reshape(TOK, HID)
    w1t = np.ascontiguousarray(w1.T)
    w2t = np.ascontiguousarray(w2.T)

    nc = build_bass()
    in_maps = [
        {
            "xt": np.ascontiguousarray(xflat[c * TPC:(c + 1) * TPC].T),
            "w1t": w1t, "b1": b1, "w2t": w2t, "b2": b2,
        }
        for c in range(NCORES)
    ]
    res = bass_utils.run_bass_kernel_spmd(nc, in_maps,
                                          core_ids=list(range(NCORES)),
                                          trace=TRACE)
    LAST_RESULT = res

    disp = np.concatenate([np.asarray(r["disp"]) for r in res.results],
                          axis=0).reshape(B, S, E, CAP)
    comb = np.concatenate([np.asarray(r["comb"]) for r in res.results],
                          axis=0).reshape(B, S, E, CAP)
    probs = np.concatenate([np.asarray(r["probs"]) for r in res.results],
                           axis=0).reshape(B, S, E)

    pbar = probs.reshape(-1, E).astype(np.float32).mean(axis=0)
    aux = np.float32(np.sum(pbar * np.log(pbar * np.float32(E) +
                                          np.float32(1e-9)),
                            dtype=np.float32))
    return disp, comb, probs, aux
